# revision 4
# baseline (speedup 1.0000x reference)
"""Trainium2 Bass kernel for the CudaFastWeightPerformerLayer problem.

Algorithm: FAVOR+ features + delta-rule fast-weight recurrence, computed with
the chunked WY/UT-transform parallel form (chunk C=128, Neumann-2 solve of the
unit-triangular system). Sharding: core c handles batch b=c%2 and the 4 heads
[4*(c//2), 4*(c//2)+4). A second small dispatch does the W_o projection,
residual add and layernorm, sharded over sequence positions.

Self-contained: all shapes hardcoded; inputs are the full unsharded tensors.
"""
import numpy as np
import ml_dtypes

SLEN, BSZ, D_MODEL, N_HEAD, D_HEAD, PROJ_DIM = 2048, 2, 1024, 16, 64, 256
LN_EPS = 1e-5
PRIME_EPS = 1e-4
P2M = 2 * PROJ_DIM          # 512 feature dim
C = 128                      # chunk length
NCHUNK = SLEN // C           # 16
HPC = 4                      # heads per core
N_CORES = 8
NEUMANN = 2

_cache = {}


def _build_phase1():
    import concourse.bacc as bacc
    import concourse.mybir as mybir
    import concourse.tile as tile

    dt = mybir.dt
    AF = mybir.ActivationFunctionType
    nc = bacc.Bacc("TRN2", target_bir_lowering=False, debug=False)

    hT = nc.dram_tensor("hT", (D_MODEL, SLEN), dt.bfloat16, kind="ExternalInput").ap()
    Wq = nc.dram_tensor("Wq", (D_MODEL, 256), dt.bfloat16, kind="ExternalInput").ap()
    Wk = nc.dram_tensor("Wk", (D_MODEL, 256), dt.bfloat16, kind="ExternalInput").ap()
    Wvb = nc.dram_tensor("Wvb", (D_MODEL, 260), dt.bfloat16, kind="ExternalInput").ap()
    pmA = nc.dram_tensor("pmA", (128, P2M), dt.bfloat16, kind="ExternalInput").ap()
    maskS = nc.dram_tensor("maskS", (128, 512), dt.float32, kind="ExternalInput").ap()
    maskI = nc.dram_tensor("maskI", (128, 512), dt.float32, kind="ExternalInput").ap()
    outs = nc.dram_tensor("outs", (SLEN, 256), dt.float32, kind="ExternalOutput").ap()

    cxn = float(D_HEAD ** -0.25)
    with tile.TileContext(nc) as tc:
        with (
            tc.tile_pool(name="const", bufs=1) as cpool,
            tc.tile_pool(name="feat", bufs=1) as fpool,
            tc.tile_pool(name="kq", bufs=8) as kqpool,
            tc.tile_pool(name="small", bufs=3) as spool,
            tc.tile_pool(name="outp", bufs=3) as opool,
            tc.tile_pool(name="ps_big", bufs=1, space="PSUM") as psb,
            tc.tile_pool(name="ps_prj", bufs=1, space="PSUM") as psprj,
            tc.tile_pool(name="ps_v", bufs=1, space="PSUM") as psv,
            tc.tile_pool(name="ps_ax", bufs=1, space="PSUM") as psax,
        ):
            # ---- load constants / weights ----
            hT_sb = cpool.tile([128, 8 * SLEN], dt.bfloat16, tag="hT")
            for t in range(8):
                nc.sync.dma_start(hT_sb[:, t * SLEN:(t + 1) * SLEN],
                                  hT[t * 128:(t + 1) * 128, :])
            Wq_sb = cpool.tile([128, 8 * 256], dt.bfloat16, tag="Wq")
            Wk_sb = cpool.tile([128, 8 * 256], dt.bfloat16, tag="Wk")
            Wvb_sb = cpool.tile([128, 8 * 260], dt.bfloat16, tag="Wvb")
            for t in range(8):
                nc.sync.dma_start(Wq_sb[:, t * 256:(t + 1) * 256], Wq[t * 128:(t + 1) * 128, :])
                nc.sync.dma_start(Wk_sb[:, t * 256:(t + 1) * 256], Wk[t * 128:(t + 1) * 128, :])
                nc.sync.dma_start(Wvb_sb[:, t * 260:(t + 1) * 260], Wvb[t * 128:(t + 1) * 128, :])
            pmA_sb = cpool.tile([128, P2M], dt.bfloat16, tag="pmA")
            nc.sync.dma_start(pmA_sb[:], pmA[:])
            maskS_sb = cpool.tile([128, 512], dt.float32, tag="maskS")
            maskI_sb = cpool.tile([128, 512], dt.float32, tag="maskI")
            nc.sync.dma_start(maskS_sb[:], maskS[:])
            nc.sync.dma_start(maskI_sb[:], maskI[:])

            # ---- phase A: xn_aug per head (128 rows = [xn(64); xn^2(64)]) ----
            xq = [fpool.tile([128, SLEN], dt.bfloat16, tag=f"xq{h}", name=f"xq{h}") for h in range(HPC)]
            xk = [fpool.tile([128, SLEN], dt.bfloat16, tag=f"xk{h}", name=f"xk{h}") for h in range(HPC)]
            for g in range(2):          # head group (2 heads)
                for lt in range(4):     # l tiles of 512
                    qps = psprj.tile([128, 512], dt.float32, tag="prj")
                    for kt in range(8):
                        nc.tensor.matmul(
                            qps[:],
                            lhsT=Wq_sb[:, kt * 256 + g * 128: kt * 256 + (g + 1) * 128],
                            rhs=hT_sb[:, kt * SLEN + lt * 512: kt * SLEN + (lt + 1) * 512],
                            start=(kt == 0), stop=(kt == 7))
                    for hh in range(2):
                        h = g * 2 + hh
                        sl = qps[hh * 64:(hh + 1) * 64, :]
                        nc.vector.tensor_scalar_mul(
                            xq[h][0:64, lt * 512:(lt + 1) * 512], sl, cxn)
                        nc.scalar.activation(
                            xq[h][64:128, lt * 512:(lt + 1) * 512], sl,
                            AF.Square, scale=cxn)
                    kps = psprj.tile([128, 512], dt.float32, tag="prj")
                    for kt in range(8):
                        nc.tensor.matmul(
                            kps[:],
                            lhsT=Wk_sb[:, kt * 256 + g * 128: kt * 256 + (g + 1) * 128],
                            rhs=hT_sb[:, kt * SLEN + lt * 512: kt * SLEN + (lt + 1) * 512],
                            start=(kt == 0), stop=(kt == 7))
                    for hh in range(2):
                        h = g * 2 + hh
                        sl = kps[hh * 64:(hh + 1) * 64, :]
                        nc.vector.tensor_scalar_mul(
                            xk[h][0:64, lt * 512:(lt + 1) * 512], sl, cxn)
                        nc.scalar.activation(
                            xk[h][64:128, lt * 512:(lt + 1) * 512], sl,
                            AF.Square, scale=cxn)

            # ---- scan state ----
            st_ps = [psb.tile([128, 512], dt.float32, tag=f"st{i}", name=f"st{i}") for i in range(2)]
            st_sb = fpool.tile([128, 1024], dt.bfloat16, tag="st_sb")
            nc.vector.memset(st_sb[:], 0.0)

            for c in range(NCHUNK):
                first = (c == 0)
                # v/beta projection for this chunk: (128 l, 260)
                vps = psv.tile([128, 260], dt.float32, tag="vps")
                for kt in range(8):
                    nc.tensor.matmul(
                        vps[:],
                        lhsT=hT_sb[:, kt * SLEN + c * 128: kt * SLEN + (c + 1) * 128],
                        rhs=Wvb_sb[:, kt * 260:(kt + 1) * 260],
                        start=(kt == 0), stop=(kt == 7))
                beta = spool.tile([128, 4], dt.float32, tag="beta")
                nc.scalar.activation(beta[:], vps[:, 256:260], AF.Sigmoid)

                # features per head
                ktm, qtm, kqfm = [], [], []
                sigk = spool.tile([128, 4], dt.float32, tag="sigk")
                sigq = spool.tile([128, 4], dt.float32, tag="sigq")
                for h in range(HPC):
                    prj = psprj.tile([128, 512], dt.float32, tag="prj")
                    nc.tensor.matmul(prj[:], lhsT=xk[h][:, c * 128:(c + 1) * 128],
                                     rhs=pmA_sb[:], start=True, stop=True)
                    kt_t = kqpool.tile([128, 512], dt.bfloat16, tag="ktm")
                    nc.scalar.activation(kt_t[:], prj[:], AF.Exp,
                                         accum_out=sigk[:, h:h + 1])
                    ktm.append(kt_t)
                    prq = psprj.tile([128, 512], dt.float32, tag="prj")
                    nc.tensor.matmul(prq[:], lhsT=xq[h][:, c * 128:(c + 1) * 128],
                                     rhs=pmA_sb[:], start=True, stop=True)
                    qt_t = kqpool.tile([128, 512], dt.bfloat16, tag="qtm")
                    nc.scalar.activation(qt_t[:], prq[:], AF.Exp,
                                         accum_out=sigq[:, h:h + 1])
                    qtm.append(qt_t)
                    fm = kqpool.tile([128, 1024], dt.bfloat16, tag="kqfm")
                    for t in range(4):
                        nc.sync.dma_start_transpose(
                            fm[:, t * 128:(t + 1) * 128],
                            kt_t[:, t * 128:(t + 1) * 128])
                        nc.sync.dma_start_transpose(
                            fm[:, 512 + t * 128: 512 + (t + 1) * 128],
                            qt_t[:, t * 128:(t + 1) * 128])
                    kqfm.append(fm)

                # per-token scalars
                skp = spool.tile([128, 4], dt.float32, tag="skp")
                nc.vector.tensor_scalar_add(skp[:], sigk[:], P2M * PRIME_EPS)
                rk = spool.tile([128, 4], dt.float32, tag="rk")
                nc.vector.reciprocal(rk[:], skp[:])
                bp = spool.tile([128, 4], dt.float32, tag="bp")
                nc.vector.tensor_mul(bp[:], rk[:], rk[:])
                nc.vector.tensor_mul(bp[:], bp[:], beta[:])
                sqp = spool.tile([128, 4], dt.float32, tag="sqp")
                nc.vector.tensor_scalar_add(sqp[:], sigq[:], P2M * PRIME_EPS)
                rq = spool.tile([128, 4], dt.float32, tag="rq")
                nc.vector.reciprocal(rq[:], sqp[:])
                nc.vector.tensor_scalar_mul(rq[:], rq[:], float(D_HEAD ** -0.5))

                # G | GQ  (per head cols h*256: [G 128 | GQ 128])
                ggq = psb.tile([128, 1024], dt.float32, tag="ggq")
                for h in range(HPC):
                    for t in range(4):
                        rhs = kqfm[h][:].rearrange(
                            "p (two x) -> p two x", two=2)[:, :, t * 128:(t + 1) * 128]
                        nc.tensor.matmul(
                            ggq[:, h * 256:(h + 1) * 256],
                            lhsT=kqfm[h][:, t * 128:(t + 1) * 128],
                            rhs=rhs,
                            start=(t == 0 and h % 2 == 0), stop=(t == 3 and h % 2 == 1))
                # masked copies: Gm (strict upper), M2 (incl upper)
                gm = spool.tile([128, 512], dt.bfloat16, tag="gm")
                m2 = spool.tile([128, 512], dt.bfloat16, tag="m2")
                g_src = ggq[:].rearrange("p (h x) -> p h x", x=256)
                nc.vector.tensor_mul(
                    gm[:].rearrange("p (h x) -> p h x", x=128),
                    g_src[:, :, 0:128],
                    maskS_sb[:].rearrange("p (h x) -> p h x", x=128))
                nc.vector.tensor_mul(
                    m2[:].rearrange("p (h x) -> p h x", x=128),
                    g_src[:, :, 128:256],
                    maskI_sb[:].rearrange("p (h x) -> p h x", x=128))

                # KS | QS(+O)
                ksqs = psb.tile([128, 512], dt.float32, tag="ksqs")
                for h in range(HPC):
                    for t in range(4):
                        nc.tensor.matmul(
                            ksqs[:, h * 64:(h + 1) * 64],
                            lhsT=kqfm[h][:, t * 128:(t + 1) * 128],
                            rhs=st_sb[:, h * 256 + t * 64: h * 256 + (t + 1) * 64],
                            start=(h == 0 and t == 0), stop=False)
                for h in range(HPC):
                    for t in range(4):
                        nc.tensor.matmul(
                            ksqs[:, 256 + h * 64: 256 + (h + 1) * 64],
                            lhsT=kqfm[h][:, 512 + t * 128: 512 + (t + 1) * 128],
                            rhs=st_sb[:, h * 256 + t * 64: h * 256 + (t + 1) * 64],
                            start=False, stop=False)

                # B = bp * (skp * v - KS)   (per head, bf16)
                bmat = spool.tile([128, 256], dt.bfloat16, tag="bmat")
                tmp1 = spool.tile([128, 256], dt.float32, tag="tmp1")
                for h in range(HPC):
                    nc.vector.tensor_scalar_mul(
                        tmp1[:, h * 64:(h + 1) * 64],
                        vps[:, h * 64:(h + 1) * 64], skp[:, h:h + 1])
                for h in range(HPC):
                    nc.vector.tensor_sub(
                        tmp1[:, h * 64:(h + 1) * 64],
                        tmp1[:, h * 64:(h + 1) * 64],
                        ksqs[:, h * 64:(h + 1) * 64])
                for h in range(HPC):
                    nc.vector.tensor_scalar_mul(
                        bmat[:, h * 64:(h + 1) * 64],
                        tmp1[:, h * 64:(h + 1) * 64], bp[:, h:h + 1])

                # Neumann: X <- B - bp*(Gm^T.T @ X)
                x_cur = bmat
                for it in range(NEUMANN):
                    ax = psax.tile([128, 256], dt.float32, tag="ax")
                    for h in range(HPC):
                        nc.tensor.matmul(
                            ax[:, h * 64:(h + 1) * 64],
                            lhsT=gm[:, h * 128:(h + 1) * 128],
                            rhs=x_cur[:, h * 64:(h + 1) * 64],
                            start=(h == 0), stop=(h == 3))
                    x_new = spool.tile([128, 256], dt.bfloat16, tag=f"x{it}")
                    for h in range(HPC):
                        nc.vector.tensor_scalar_mul(
                            tmp1[:, h * 64:(h + 1) * 64],
                            ax[:, h * 64:(h + 1) * 64], bp[:, h:h + 1])
                    nc.vector.tensor_sub(x_new[:], bmat[:], tmp1[:])
                    x_cur = x_new

                # O += tril(QK^T,0) @ U   (accumulate onto QS half of ksqs)
                for h in range(HPC):
                    nc.tensor.matmul(
                        ksqs[:, 256 + h * 64: 256 + (h + 1) * 64],
                        lhsT=m2[:, h * 128:(h + 1) * 128],
                        rhs=x_cur[:, h * 64:(h + 1) * 64],
                        start=False, stop=(h == 3))
                # out = O * rq
                o_sb = opool.tile([128, 256], dt.float32, tag="o_sb")
                for h in range(HPC):
                    nc.vector.tensor_scalar_mul(
                        o_sb[:, h * 64:(h + 1) * 64],
                        ksqs[:, 256 + h * 64: 256 + (h + 1) * 64], rq[:, h:h + 1])
                nc.sync.dma_start(outs[c * 128:(c + 1) * 128, :], o_sb[:])

                # S update: st += K^T @ U ; refresh st_sb (bf16)
                for h in range(HPC):
                    for t in range(4):
                        nc.tensor.matmul(
                            st_ps[h // 2][:, (h % 2) * 256 + t * 64: (h % 2) * 256 + (t + 1) * 64],
                            lhsT=ktm[h][:, t * 128:(t + 1) * 128],
                            rhs=x_cur[:, h * 64:(h + 1) * 64],
                            start=(first and h % 2 == 0 and t == 0), stop=False)
                if c < NCHUNK - 1:
                    nc.vector.tensor_copy(st_sb[:, 0:512], st_ps[0][:])
                    nc.vector.tensor_copy(st_sb[:, 512:1024], st_ps[1][:])
    nc.compile()
    return nc


def _build_phase2():
    import concourse.bacc as bacc
    import concourse.mybir as mybir
    import concourse.tile as tile

    dt = mybir.dt
    AF = mybir.ActivationFunctionType
    nc = bacc.Bacc("TRN2", target_bir_lowering=False, debug=False)
    R = SLEN // N_CORES  # 256 rows per core per batch

    oT0 = nc.dram_tensor("oT0", (D_MODEL, R), dt.bfloat16, kind="ExternalInput").ap()
    oT1 = nc.dram_tensor("oT1", (D_MODEL, R), dt.bfloat16, kind="ExternalInput").ap()
    h0 = nc.dram_tensor("h0", (R, D_MODEL), dt.float32, kind="ExternalInput").ap()
    h1 = nc.dram_tensor("h1", (R, D_MODEL), dt.float32, kind="ExternalInput").ap()
    Wo = nc.dram_tensor("Wo", (D_MODEL, D_MODEL), dt.bfloat16, kind="ExternalInput").ap()
    gam = nc.dram_tensor("gam", (128, D_MODEL), dt.float32, kind="ExternalInput").ap()
    bet = nc.dram_tensor("bet", (128, D_MODEL), dt.float32, kind="ExternalInput").ap()
    y = nc.dram_tensor("y", (2 * R, D_MODEL), dt.float32, kind="ExternalOutput").ap()

    with tile.TileContext(nc) as tc:
        with (
            tc.tile_pool(name="const", bufs=1) as cpool,
            tc.tile_pool(name="work", bufs=3) as wpool,
            tc.tile_pool(name="ps", bufs=2, space="PSUM") as ps,
        ):
            wo_sb = cpool.tile([128, 8 * D_MODEL], dt.bfloat16, tag="wo")
            for t in range(8):
                nc.sync.dma_start(wo_sb[:, t * D_MODEL:(t + 1) * D_MODEL],
                                  Wo[t * 128:(t + 1) * 128, :])
            gam_sb = cpool.tile([128, D_MODEL], dt.float32, tag="gam")
            bet_sb = cpool.tile([128, D_MODEL], dt.float32, tag="bet")
            nc.sync.dma_start(gam_sb[:], gam[:])
            nc.sync.dma_start(bet_sb[:], bet[:])
            oT_sb = [cpool.tile([128, 8 * R], dt.bfloat16, tag=f"oT{b}", name=f"oT{b}") for b in range(2)]
            for b, src in ((0, oT0), (1, oT1)):
                for t in range(8):
                    nc.sync.dma_start(oT_sb[b][:, t * R:(t + 1) * R],
                                      src[t * 128:(t + 1) * 128, :])

            for b, hsrc in ((0, h0), (1, h1)):
                for lt in range(R // 128):  # 2 l-tiles of 128
                    h_sb = wpool.tile([128, D_MODEL], dt.float32, tag="h_sb")
                    nc.sync.dma_start(h_sb[:], hsrc[lt * 128:(lt + 1) * 128, :])
                    x_sb = wpool.tile([128, D_MODEL], dt.float32, tag="x_sb")
                    for nt in range(2):  # output col halves of 512
                        acc = ps.tile([128, 512], dt.float32, tag="acc")
                        for kt in range(8):
                            nc.tensor.matmul(
                                acc[:],
                                lhsT=oT_sb[b][:, kt * R + lt * 128: kt * R + (lt + 1) * 128],
                                rhs=wo_sb[:, kt * D_MODEL + nt * 512: kt * D_MODEL + (nt + 1) * 512],
                                start=(kt == 0), stop=(kt == 7))
                        nc.vector.tensor_add(
                            x_sb[:, nt * 512:(nt + 1) * 512],
                            h_sb[:, nt * 512:(nt + 1) * 512], acc[:])
                    # layernorm over free dim (1024)
                    ssum = wpool.tile([128, 1], dt.float32, tag="ssum")
                    nc.vector.reduce_sum(ssum[:], x_sb[:], axis=mybir.AxisListType.X)
                    sqa = wpool.tile([128, 1], dt.float32, tag="sqa")
                    dummy = wpool.tile([128, D_MODEL], dt.float32, tag="dummy")
                    nc.scalar.activation(dummy[:], x_sb[:], AF.Square,
                                         accum_out=sqa[:])
                    mu = wpool.tile([128, 1], dt.float32, tag="mu")
                    nc.vector.tensor_scalar_mul(mu[:], ssum[:], 1.0 / D_MODEL)
                    # var = E[x^2] - mu^2
                    mu2 = wpool.tile([128, 1], dt.float32, tag="mu2")
                    nc.vector.tensor_mul(mu2[:], mu[:], mu[:])
                    var = wpool.tile([128, 1], dt.float32, tag="var")
                    nc.vector.tensor_scalar_mul(var[:], sqa[:], 1.0 / D_MODEL)
                    nc.vector.tensor_sub(var[:], var[:], mu2[:])
                    nc.vector.tensor_scalar_add(var[:], var[:], LN_EPS)
                    rstd = wpool.tile([128, 1], dt.float32, tag="rstd")
                    nc.scalar.activation(rstd[:], var[:], AF.Sqrt)
                    nc.vector.reciprocal(rstd[:], rstd[:])
                    nmu = wpool.tile([128, 1], dt.float32, tag="nmu")
                    nc.vector.tensor_mul(nmu[:], mu[:], rstd[:])
                    nc.vector.tensor_scalar_mul(nmu[:], nmu[:], -1.0)
                    xs = wpool.tile([128, D_MODEL], dt.float32, tag="xs")
                    nc.vector.tensor_scalar(xs[:], x_sb[:], rstd[:], nmu[:],
                                            op0=mybir.AluOpType.mult,
                                            op1=mybir.AluOpType.add)
                    # gamma/beta broadcast along partitions
                    nc.vector.tensor_mul(xs[:], xs[:], gam_sb[:])
                    nc.vector.tensor_add(xs[:], xs[:], bet_sb[:])
                    nc.sync.dma_start(y[b * R + lt * 128: b * R + (lt + 1) * 128, :], xs[:])
    nc.compile()
    return nc


def _run(nc, in_maps):
    from concourse.bass_utils import run_bass_kernel_spmd
    res = run_bass_kernel_spmd(nc, in_maps, core_ids=list(range(N_CORES)))
    return res.results


def kernel(h, W_qkvb, W_o, ln_gamma, ln_beta, proj_matrix):
    bf16 = ml_dtypes.bfloat16
    h = np.asarray(h, np.float32)
    Wr = np.asarray(W_qkvb, np.float32).reshape(D_MODEL, N_HEAD, 3 * D_HEAD + 1)
    pm = np.asarray(proj_matrix, np.float32)

    # constants
    pmA = np.zeros((128, P2M), np.float32)
    pmA[0:64, 0:256] = pm
    pmA[0:64, 256:512] = -pm
    pmA[64:128, :] = -0.5
    triuS = np.triu(np.ones((128, 128), np.float32), 1)
    triuI = np.triu(np.ones((128, 128), np.float32), 0)
    maskS = np.tile(triuS, (1, 4))
    maskI = np.tile(triuI, (1, 4))

    if "p1" not in _cache:
        _cache["p1"] = _build_phase1()
    in_maps = []
    for c in range(N_CORES):
        b = c % 2
        hb0 = 4 * (c // 2)
        hT = np.ascontiguousarray(h[:, b, :].T).astype(bf16)
        Wq = np.ascontiguousarray(Wr[:, hb0:hb0 + 4, 0:64].reshape(D_MODEL, 256)).astype(bf16)
        Wk = np.ascontiguousarray(Wr[:, hb0:hb0 + 4, 64:128].reshape(D_MODEL, 256)).astype(bf16)
        Wvb = np.concatenate([
            Wr[:, hb0:hb0 + 4, 128:192].reshape(D_MODEL, 256),
            Wr[:, hb0:hb0 + 4, 192],
        ], axis=1).astype(bf16)
        in_maps.append({
            "hT": hT, "Wq": Wq, "Wk": Wk, "Wvb": np.ascontiguousarray(Wvb),
            "pmA": pmA.astype(bf16), "maskS": maskS, "maskI": maskI,
        })
    res1 = _run(_cache["p1"], in_maps)

    # assemble outs per batch: (2048, 1024) head-major cols
    outs_b = [np.concatenate([res1[c]["outs"] for c in range(N_CORES) if c % 2 == b],
                             axis=1) for b in range(2)]

    if "p2" not in _cache:
        _cache["p2"] = _build_phase2()
    R = SLEN // N_CORES
    oT = [np.ascontiguousarray(ob.T).astype(bf16) for ob in outs_b]
    Wo_b = np.asarray(W_o, np.float32).astype(bf16)
    gam = np.tile(np.asarray(ln_gamma, np.float32).reshape(1, D_MODEL), (128, 1))
    bet = np.tile(np.asarray(ln_beta, np.float32).reshape(1, D_MODEL), (128, 1))
    in_maps2 = []
    for c in range(N_CORES):
        sl = slice(c * R, (c + 1) * R)
        in_maps2.append({
            "oT0": np.ascontiguousarray(oT[0][:, sl]),
            "oT1": np.ascontiguousarray(oT[1][:, sl]),
            "h0": np.ascontiguousarray(h[sl, 0, :]),
            "h1": np.ascontiguousarray(h[sl, 1, :]),
            "Wo": Wo_b, "gam": gam, "bet": bet,
        })
    res2 = _run(_cache["p2"], in_maps2)

    out = np.empty((SLEN, BSZ, D_MODEL), np.float32)
    for c in range(N_CORES):
        sl = slice(c * R, (c + 1) * R)
        out[sl, 0, :] = res2[c]["y"][0:R]
        out[sl, 1, :] = res2[c]["y"][R:2 * R]
    return out
